# revision 4
# baseline (speedup 1.0000x reference)
"""GQA (= full MHA) attention kernel for 8 Trainium2 NeuronCores.

Problem: B=2, T=2048 queries, K=2048 keys, H=16 heads, D=128, fp32.
The reference's "group" reshape is a no-op view: this is plain softmax
attention per (batch, head). 32 independent (b,h) problems -> 4 per core.

Per-core device program (SPMD, different input slices per core):
  - Host pre-transposes Q,K to (d, t)/(d, k) layout and casts to bf16.
  - S^T tile = K_j^T.T @ Q^T  (k on partitions, t free), 128x512 tiles.
  - P^T = exp(S^T / sqrt(D)) on the scalar engine (ACT), bf16 out.
  - O^T += V_j.T @ P^T_j  accumulated over 16 k-tiles in PSUM (fp32).
  - l = ones.T @ P^T_j accumulated (softmax denominator), via PE.
  - Device returns unnormalized O^T (d, t) fp32 and l; the host does the
    final transpose back to (t, d) and the divide by l (cheap numpy).
"""

import numpy as np
import ml_dtypes

import concourse.bacc as bacc
import concourse.tile as tile
import concourse.mybir as mybir
from concourse.bass_utils import run_bass_kernel_spmd

B = 2
T = 2048
KSEQ = 2048
H = 16
D = 128
N_CORES = 8
PAIRS = (B * H) // N_CORES  # 4 (b,h) pairs per core
TSLICE = 512
NS = T // TSLICE  # 4
KTILES = KSEQ // 128  # 16
SCALE = 1.0 / float(np.sqrt(D))

f32 = mybir.dt.float32
f16 = mybir.dt.float16

_cache = {}


def _build(repeat=1, dyn_loop=1):
    key = ("nc", repeat, dyn_loop)
    if key in _cache:
        return _cache[key]
    nc = bacc.Bacc(None, target_bir_lowering=False)
    with tile.TileContext(nc) as tc:
        with tc.tile_pool(name="dram", bufs=1, space="DRAM") as dram:
            qt_in = dram.tile([PAIRS, 128, T], f16, kind="ExternalInput",
                              name="qt_in", uniquify=False)
            kt_in = dram.tile([PAIRS, 128, KSEQ], f16, kind="ExternalInput",
                              name="kt_in", uniquify=False)
            v_in = dram.tile([PAIRS, KSEQ, D], f16, kind="ExternalInput",
                             name="v_in", uniquify=False)
            ot_out = dram.tile([PAIRS, D, T], f32, kind="ExternalOutput",
                               name="ot_out", uniquify=False)
            l_out = dram.tile([PAIRS, T], f32, kind="ExternalOutput",
                              name="l_out", uniquify=False)
            if dyn_loop > 1:
                with tc.For_i(0, dyn_loop, 1):
                    _attn_body(nc, tc, qt_in, kt_in, v_in, ot_out, l_out,
                               repeat)
            else:
                _attn_body(nc, tc, qt_in, kt_in, v_in, ot_out, l_out, repeat)
    nc.compile()
    _cache[key] = nc
    return nc


def _attn_body(nc, tc, qt_in, kt_in, v_in, ot_out, l_out, repeat):
    with (
        tc.tile_pool(name="const", bufs=1) as constp,
        tc.tile_pool(name="qkv", bufs=2) as qkv,
        tc.tile_pool(name="ptp", bufs=2) as ptp,
        tc.tile_pool(name="drain", bufs=3) as drp,
        tc.tile_pool(name="ps_s", bufs=4, space="PSUM") as ps_s,
        tc.tile_pool(name="ps_o", bufs=2, space="PSUM") as ps_o,
        tc.tile_pool(name="ps_l", bufs=2, space="PSUM") as ps_l,
    ):
        ones = constp.tile([128, 1], f16)
        nc.gpsimd.memset(ones[:], 1.0)

        def load_pair(p):
            qt = qkv.tile([128, T], f16, tag="qt", name=f"qt_{p}")
            kt = qkv.tile([128, KSEQ], f16, tag="kt", name=f"kt_{p}")
            v = qkv.tile([128, KTILES * D], f16, tag="v", name=f"v_{p}")
            nc.sync.dma_start(out=qt[:], in_=qt_in[p])
            nc.sync.dma_start(out=kt[:], in_=kt_in[p])
            for j in range(KTILES):
                nc.sync.dma_start(out=v[:, j * D:(j + 1) * D],
                                  in_=v_in[p, j * 128:(j + 1) * 128, :])
            return qt, kt, v

        # Software pipeline by one t-slice: slice i's S-matmuls + exp run
        # interleaved with slice i-1's PV / denominator matmuls, so the
        # scalar engine (exp) stays saturated under the PE's matmul stream.
        slices = [(p, s) for _ in range(repeat)
                  for p in range(PAIRS) for s in range(NS)]
        pair_tiles = {}
        prev = None
        for idx in range(len(slices) + 1):
            cur = None
            if idx < len(slices):
                p, s = slices[idx]
                if s == 0:
                    pair_tiles[p] = load_pair(p)
                qt, kt, v = pair_tiles[p]
                ts = slice(s * TSLICE, (s + 1) * TSLICE)
                pt = ptp.tile([128, KTILES * TSLICE], f16, tag="pt",
                              name=f"pt_{idx}")
                cur = (p, s, pt, v)
            if prev is not None:
                po = ps_o.tile([128, TSLICE], f32, tag="o", name=f"po_{idx}")
                pl = ps_l.tile([1, TSLICE], f32, tag="l", name=f"pl_{idx}")
            for j in range(KTILES):
                if cur is not None:
                    ps = ps_s.tile([128, TSLICE], f32, tag="s",
                                   name=f"ps_{idx}_{j}")
                    nc.tensor.matmul(
                        ps[:], kt[:, j * 128:(j + 1) * 128], qt[:, ts],
                        start=True, stop=True)
                    nc.scalar.activation(
                        pt[:, j * TSLICE:(j + 1) * TSLICE], ps[:],
                        mybir.ActivationFunctionType.Exp, scale=SCALE)
                if prev is not None:
                    pp, ss, ppt, pv = prev
                    nc.tensor.matmul(
                        po[:], pv[:, j * D:(j + 1) * D],
                        ppt[:, j * TSLICE:(j + 1) * TSLICE],
                        start=(j == 0), stop=(j == KTILES - 1))
            if prev is not None:
                pp, ss, ppt, pv = prev
                for j in range(KTILES):
                    nc.tensor.matmul(
                        pl[:], ones[:],
                        ppt[:, j * TSLICE:(j + 1) * TSLICE],
                        start=(j == 0), stop=(j == KTILES - 1))
                pts = slice(ss * TSLICE, (ss + 1) * TSLICE)
                osb = drp.tile([128, TSLICE], f32, tag="osb",
                               name=f"osb_{idx}")
                lsb = drp.tile([1, TSLICE], f32, tag="lsb", name=f"lsb_{idx}")
                nc.vector.tensor_copy(osb[:], po[:])
                nc.vector.tensor_copy(lsb[:], pl[:])
                nc.sync.dma_start(out=ot_out[pp, :, pts], in_=osb[:])
                nc.sync.dma_start(out=l_out[pp:pp + 1, pts], in_=lsb[:])
            prev = cur


def _prep(query, key, value):
    """Host-side shard + layout + cast. Returns per-core input maps."""
    q4 = query.reshape(B, T, H, D)
    # (b,h,d,t) so each pair's Q^T is (128, T) with d on partitions
    qT = np.ascontiguousarray(q4.transpose(0, 2, 3, 1)).reshape(B * H, D, T)
    kT = np.ascontiguousarray(key.transpose(0, 2, 3, 1)).reshape(B * H, D, KSEQ)
    v = np.ascontiguousarray(value.transpose(0, 2, 1, 3)).reshape(B * H, KSEQ, D)
    qT = qT.astype(np.float16)
    kT = kT.astype(np.float16)
    v = v.astype(np.float16)
    in_maps = []
    for c in range(N_CORES):
        sl = slice(c * PAIRS, (c + 1) * PAIRS)
        in_maps.append({
            "qt_in": np.ascontiguousarray(qT[sl]),
            "kt_in": np.ascontiguousarray(kT[sl]),
            "v_in": np.ascontiguousarray(v[sl]),
        })
    return in_maps


def _post(results):
    """Gather per-core outputs, normalize, restore (B, T, H*D) fp32."""
    ot = np.stack([r["ot_out"] for r in results])  # (8, PAIRS, D, T)
    l = np.stack([r["l_out"] for r in results])    # (8, PAIRS, T)
    ot = ot.reshape(B * H, D, T)
    l = l.reshape(B * H, T)
    o = ot.transpose(0, 2, 1) / l[:, :, None]      # (BH, T, D)
    o = o.reshape(B, H, T, D).transpose(0, 2, 1, 3).reshape(B, T, H * D)
    return np.ascontiguousarray(o.astype(np.float32))


def kernel(query, key, value):
    nc = _build()
    in_maps = _prep(query, key, value)
    res = run_bass_kernel_spmd(nc, in_maps, core_ids=list(range(N_CORES)))
    return _post(res.results)


if __name__ == "__main__":
    rng = np.random.default_rng(0)
    q = rng.standard_normal((B, T, H * D), dtype=np.float32)
    k = rng.standard_normal((B, KSEQ, H, D), dtype=np.float32)
    v = rng.standard_normal((B, KSEQ, H, D), dtype=np.float32)
    out = kernel(q, k, v)
    print("out", out.shape, out.dtype)
